# revision 26
# baseline (speedup 1.0000x reference)
"""AlignedTripletLoss Trainium2 kernel (8 NeuronCores, symmetric block-pair
decomposition, single-table activation + diagonal-wavefront DTW).

Math (matches reference.py within ~6e-3 rel on the final loss):
  x_hat = x / ||x||_2 per (image, part) row                        [1024*8, 128]
  s[(a,i),(b,j)] = <x_hat_(a,i), x_hat_(b,j)>;  u = 2 - 2 s  in [0, 4]
  t = tanh(sqrt(u)/2)  ~=  GAM * ln(AFIT*u + BFIT) + DLT   (max err 4.7e-4 on
      the data support u in [0.98, 3.08]; every monotone DTW path over the 8x8
      grid has exactly 15 cells, so the affine (GAM, DLT) commutes with the
      path sum and the hard mining -- the host applies GAM once at the end and
      DLT cancels in ap - an).  This collapses the former sqrt+tanh two-table
      activation chain into ONE table lookup; the norm chain uses only
      square/sqrt/copy so the whole kernel performs 2 ACT table loads.
  dtw[a,b] = monotone (right/down) min-path over the 8x8 grid of t'
  ap = max over positives, an = min over negatives,
  loss = mean(relu(GAM*(ap-an) + 0.3))

Sharding: same circulant block cover as the baseline -- core k owns anchor
block k and computes blocks (k, (k+d) mod 8), d = 0..4 (640 columns per
core); row mining locally, column mining of off-diagonal blocks after a PE
transpose, 10 partial min/max vectors per core combined on the host.

DTW engine strategy: tensor_tensor_scan has no DVE fast modes (~1.8 ns/elem
measured) and the Pool engine rejects scan/tensor_tensor/PSUM opcodes at
codegen, so the DP runs as an anti-diagonal wavefront in fp16 on the DVE,
where tensor_tensor qualifies for the 2x_1p perf mode (~0.62 ns/elem
measured).  Layout (all verified against CoreSim and the HW numerics):
  - t' slabs ta0/ta1 [i][j][b] (one tile per pipeline slot, so batch n+1's
    Ln writes do not falsely serialize against batch n's DP reads under the
    coarse per-tile dependency tracking);
  - DP values dpa[dpi=d-1][c=i+1][b]: val(d,c) = t'(i,d-i) +
    min(val(d-1,c-1), val(d-1,c)), two tensor_tensor ops per diagonal over
    the valid cells only; c=0 and the one read-invalid top slot per diagonal
    hold +30000 sentinels that a single strided memset initializes for the
    whole run (the DP writes the exact same cell set every batch);
  - diag 1 is a single broadcast-add of t'(0,0);
  - column strides padded to CBMAX+8 to stay off power-of-2 SBUF strides.
Per-core hard-mining partials flow out as in the baseline; the fused
tensor_tensor_reduce is avoided (it faults on HW with fp16 operands).

Matmuls run in bf16 (measured ~2x faster than f32r per column), 4/2 chained
512-free matmuls per (i, batch); the Ln activation reads PSUM in one call
per (i, batch) and writes the fp16 t' slab directly.
"""

import numpy as np

N, M, D = 1024, 8, 128
MARGIN = 0.3
NCORES = 8
A = N // NCORES          # anchors per core (one image block)
NDIAG = 5                # circulant depth: blocks k..k+4
NCOL = NDIAG * A         # 640 columns per core
CBS = [256, 256, 128]    # columns per batch
CBMAX = 256
NB = len(CBS)
ND = 15                  # wavefront diagonals
NC9 = 9                  # cell slots per diagonal (c=0 sentinel + i=0..7)
SENT = 30000.0           # fp16 sentinel (never summed with valid cells)
MBIG = 30000.0           # fp16 mining mask magnitude

# t = tanh(sqrt(u)/2) ~= GAM*ln(AFIT*u + BFIT) + DLT on u in [0.65, 3.45]
AFIT = 0.3925214
BFIT = 0.0875143
GAM = 0.2450909
# ACT input is p = -2*s = u - 2  ->  ln(AFIT*p + (2*AFIT + BFIT))
ACT_SCALE = AFIT
ACT_BIAS = 2.0 * AFIT + BFIT

_CACHE = {}


def _build_nc():
    import concourse.bacc as bacc
    import concourse.mybir as mybir
    import concourse.tile as tile
    from concourse.masks import make_identity

    fp32 = mybir.dt.float32
    bf16 = mybir.dt.bfloat16
    fp16 = mybir.dt.float16
    AF = mybir.ActivationFunctionType
    OP = mybir.AluOpType
    AX = mybir.AxisListType

    nc = bacc.Bacc("TRN2", target_bir_lowering=False, debug=False,
                   num_devices=NCORES)

    xa_in = nc.dram_tensor("xa", [A * M, D], fp32, kind="ExternalInput")
    xr_in = nc.dram_tensor("xr5", [NCOL * M, D], fp32, kind="ExternalInput")
    mop_in = nc.dram_tensor("m_own_pos", [A, NCOL], fp32, kind="ExternalInput")
    mon_in = nc.dram_tensor("m_own_neg", [A, NCOL], fp32, kind="ExternalInput")
    mtp_in = nc.dram_tensor("m_t_pos", [A, (NDIAG - 1) * A], fp32,
                            kind="ExternalInput")
    mtn_in = nc.dram_tensor("m_t_neg", [A, (NDIAG - 1) * A], fp32,
                            kind="ExternalInput")
    out_t = nc.dram_tensor("partials", [A, 10], fp32, kind="ExternalOutput")

    S = (NCOL * M) // 128   # 40 row-tiles of xr5
    SA = (A * M) // 128     # 8 row-tiles of xa

    col0s = [sum(CBS[:i]) for i in range(NB)]
    # batch -> off-diagonal blocks whose 128 columns complete at that batch
    blk_done = {nb: [] for nb in range(NB)}
    for d in range(1, NDIAG):
        last_col = (d + 1) * A - 1
        blk_done[next(i for i in range(NB)
                      if col0s[i] + CBS[i] > last_col)].append(d)

    CBP = CBMAX + 8   # padded column stride (stays off power-of-2 strides)

    with tile.TileContext(nc) as tc:
        with tc.tile_pool(name="persist", bufs=1) as persist:
            xrT = persist.tile([128, NCOL, M], bf16)   # x_hat^T [d][b][j]
            xTa = persist.tile([128, M, A], bf16)      # -2*x_hat_a^T [d][i][a]
            mop = persist.tile([128, NCOL], fp32)
            mon = persist.tile([128, NCOL], fp32)
            mtp = persist.tile([128, (NDIAG - 1) * A], fp32)
            mtn = persist.tile([128, (NDIAG - 1) * A], fp32)
            biasT = persist.tile([128, 1], fp32)
            dtwc = persist.tile([128, NCOL], fp32)     # compact dtw' (fp32)
            apacc = persist.tile([128, NB], fp32)
            anacc = persist.tile([128, NB], fp32)
            pout = persist.tile([128, 10], fp32)
            ident = persist.tile([128, 128], fp32)
            # t' slabs, one tile per pipeline slot so batch n+1's activations
            # don't falsely serialize against batch n's DP reads; packed
            # [i][j][b] cell layout (all 64 cells written by ACT).
            ta0 = persist.tile([128, M, M, CBP], fp16)
            ta1 = persist.tile([128, M, M, CBP], fp16)
            # DP wavefront values: slot dpi = d-1 holds diag d's cells at
            # c = i+1 (c=0 sentinel); written cells are the same every batch,
            # so the sentinel slots need one memset for the whole run.
            dpa = persist.tile([128, ND - 1, NC9, CBP], fp16)

            # inputs first (the norm chain gates everything), masks last
            # (only read by mining, >50us in)
            nc.gpsimd.memset(biasT[:], ACT_BIAS)
            # dummy op so the first ACT table load picks the sqrt set (the
            # per-first-use chooser would otherwise load a square-only set
            # and reload for Sqrt mid-norm-chain)
            nc.scalar.activation(pout[:, 0:1], biasT[:], AF.Sqrt)
            make_identity(nc, ident[:])

            # ---------- setup: normalize + transpose (scale fused) ---------
            with (
                tc.tile_pool(name="setup", bufs=1) as setup,
                tc.tile_pool(name="chunk", bufs=2) as chunk,
                tc.tile_pool(name="dgp", bufs=3) as dgp,
                tc.tile_pool(name="tpsum", bufs=2, space="PSUM") as tpsum,
            ):
                def norm_rn(src_dram, n_tiles, neg2, tagp, src_ap=None):
                    """Rows p-outer (row r = p*n_tiles + s); rn = 1/||row||."""
                    xr = setup.tile([128, n_tiles, D], fp32, tag=f"xr{tagp}")
                    if src_ap is None:
                        src_ap = src_dram.rearrange("(p s) d -> p s d", p=128)
                        # split across DMA queues so the load lands fast
                        for t0 in range(0, n_tiles, 2):
                            nc.sync.dma_start(
                                xr[:, t0:t0 + 2, :], src_ap[:, t0:t0 + 2, :])
                    else:
                        xrv = xr.rearrange("p (blk s) d -> p blk s d", s=M)
                        nblk = n_tiles // M
                        for b0 in range(0, nblk, 2):
                            b1 = min(b0 + 2, nblk)
                            nc.sync.dma_start(
                                xrv[:, b0:b1, :, :], src_ap[:, b0:b1, :, :])
                    n2 = setup.tile([128, n_tiles], fp32, tag=f"n2{tagp}")
                    rn = setup.tile([128, n_tiles], fp32, tag=f"rn{tagp}")
                    for g in range(0, n_tiles, 16):
                        CH = min(16, n_tiles - g)
                        x2 = chunk.tile([128, 16, D], fp32, tag="x2c")
                        # square on the DVE: it is idle through setup while
                        # the ACT path gates the first main Ln call
                        nc.vector.tensor_tensor(
                            x2[:, :CH, :], xr[:, g:g + CH, :],
                            xr[:, g:g + CH, :], OP.mult)
                        nc.vector.tensor_reduce(
                            n2[:, g:g + CH], x2[:, :CH, :], axis=AX.X, op=OP.add)
                        # square/sqrt/copy share one table set; the main Ln
                        # is the only other set -> 2 ACT table loads total.
                        nc.scalar.activation(
                            n2[:, g:g + CH], n2[:, g:g + CH], AF.Sqrt)
                        nc.vector.reciprocal(
                            rn[:, g:g + CH], n2[:, g:g + CH])
                        if neg2:
                            nc.vector.tensor_scalar_mul(
                                rn[:, g:g + CH], rn[:, g:g + CH], -2.0)
                    return xr, rn

                def diag4(rn, s0):
                    dgc = dgp.tile([128, 4, 128], fp32, tag="dgc")
                    for jj in range(4):
                        nc.gpsimd.affine_select(
                            out=dgc[:, jj, :],
                            in_=rn[:, s0 + jj:s0 + jj + 1].to_broadcast((128, 128)),
                            compare_op=OP.is_equal, fill=0.0, base=0,
                            pattern=[[-1, 128]], channel_multiplier=1)
                    return dgc

                xra, rna = norm_rn(xa_in, SA, neg2=True, tagp="a")
                for half in range(2):
                    dgc = diag4(rna, 4 * half)
                    pt = tpsum.tile([128, 4, 128], fp32, tag="tp")
                    for jj in range(4):
                        s = 4 * half + jj
                        nc.tensor.matmul(
                            pt[:, jj, :], lhsT=xra[:, s, :],
                            rhs=dgc[:, jj, :], start=True, stop=True)
                    # tile s holds rows r = p*8+s -> (a=p, i=s)
                    dst = xTa[:, 4 * half:4 * half + 4, :]
                    nc.scalar.activation(dst, pt[:], AF.Copy)

                # xr5 per block: tile t = blk*8 + s holds rows
                # blk*1024 + p*8 + s -> (col = blk*128 + p, j = s)
                xr, rn = norm_rn(
                    xr_in, S, neg2=False, tagp="x",
                    src_ap=xr_in.rearrange(
                        "(blk p s) d -> p blk s d", p=128, s=M))
                for blk in range(NDIAG):
                    for half in range(2):
                        dgc = diag4(rn, 8 * blk + 4 * half)
                        pt = tpsum.tile([128, 4, 128], fp32, tag="tp")
                        for jj in range(4):
                            s = 8 * blk + 4 * half + jj
                            nc.tensor.matmul(
                                pt[:, jj, :], lhsT=xr[:, s, :],
                                rhs=dgc[:, jj, :], start=True, stop=True)
                        dst = xrT[:, blk * A:(blk + 1) * A,
                                  4 * half:4 * half + 4]
                        nc.scalar.activation(
                            dst, pt.rearrange("d j b -> d b j"), AF.Copy)

            # masks load while the setup computes (mining reads them late)
            nc.sync.dma_start(mop[:], mop_in[:])
            nc.sync.dma_start(mon[:], mon_in[:])
            nc.sync.dma_start(mtp[:], mtp_in[:])
            nc.sync.dma_start(mtn[:], mtn_in[:])
            # dpa sentinels actually read by the DP: slot c=0 of dpi=0..5
            # (flat (dpi,c) stride 10) and the top cell (dpi, dpi+3)
            # (flat stride 11); written DP cells are identical every batch.
            dpf = dpa.rearrange("p d c b -> p (d c) b")
            nc.gpsimd.memset(dpf[:, 0:46:9, :], SENT)
            nc.gpsimd.memset(dpf[:, 3:54:10, :], SENT)

            # ---------- main loop ----------
            with (
                tc.tile_pool(name="mbp", bufs=2) as mbp,
                tc.tile_pool(name="mtmp", bufs=4) as mtmp,
                tc.tile_pool(name="mpsum", bufs=2, space="PSUM") as mpsum,
            ):
                tslabs = [ta0, ta1]

                def emit_post(n):
                    """dtwc copy + transposed-block mining for batch n."""
                    CB = CBS[n]
                    col0 = col0s[n]
                    nc.scalar.activation(
                        dtwc[:, col0:col0 + CB], dpa[:, ND - 2, 8, :CB],
                        AF.Copy)
                    for d in blk_done[n]:
                        ptt = mpsum.tile([128, CBMAX, M], fp32, tag="pp")
                        ptp = ptt.rearrange("p c j -> p (c j)")[:, :128]
                        nc.tensor.transpose(
                            ptp, dtwc[:, d * A:(d + 1) * A], ident[:])
                        tb = mtmp.tile([128, 128], fp32, tag="tb")
                        nc.vector.tensor_copy(tb[:], ptp)
                        scr2 = mtmp.tile([128, 128], fp32, tag="sc2")
                        nc.vector.tensor_tensor(
                            scr2[:], tb[:], mtp[:, (d - 1) * A:d * A], OP.add)
                        nc.vector.tensor_reduce(
                            pout[:, 2 * d + 1:2 * d + 2], scr2[:],
                            axis=AX.X, op=OP.max)
                        nc.vector.tensor_tensor(
                            scr2[:], tb[:], mtn[:, (d - 1) * A:d * A], OP.add)
                        nc.vector.tensor_reduce(
                            pout[:, 2 * d:2 * d + 1], scr2[:],
                            axis=AX.X, op=OP.min)

                for n in range(NB):
                    CB = CBS[n]
                    col0 = col0s[n]
                    ta = tslabs[n % 2]
                    taf = ta.rearrange("p i j b -> p (i j) b")
                    for i in range(M):
                        pp = mpsum.tile([128, CBMAX, M], fp32, tag="pp")
                        for q0 in range(0, CB, 64):
                            nc.tensor.matmul(
                                pp[:, q0:q0 + 64, :], lhsT=xTa[:, i, :],
                                rhs=xrT[:, col0 + q0:col0 + q0 + 64, :],
                                start=True, stop=True)
                        nc.scalar.activation(
                            ta[:, i, :, :CB],
                            pp[:, :CB, :].rearrange("p c j -> p j c"),
                            AF.Ln, bias=biasT[:, 0:1], scale=ACT_SCALE)

                    # deferred post-processing of the previous batch (keeps
                    # the ACT queue free for this batch's Ln calls)
                    if n > 0:
                        emit_post(n - 1)

                    # anti-diagonal DP, fp16 tensor_tensor (2x_1p).
                    # val(0,0) = t'(0,0); diag 1 adds it to t'(0,1)/t'(1,0).
                    nc.vector.tensor_tensor(
                        dpa[:, 0, 1:3, :CB], taf[:, 1:9:7, :CB],
                        taf[:, 0:1, :CB].to_broadcast((128, 2, CB)), OP.add)
                    for d in range(2, ND):
                        lo, hi = max(0, d - 7), min(7, d)
                        c0, c1 = lo + 1, hi + 1
                        ncl = hi - lo + 1
                        mb = mbp.tile([128, 8, CBP], fp16, tag="mb")
                        nc.vector.tensor_tensor(
                            mb[:, :ncl, :CB],
                            dpa[:, d - 2, c0 - 1:c1, :CB],
                            dpa[:, d - 2, c0:c1 + 1, :CB], OP.min)
                        nc.vector.tensor_tensor(
                            dpa[:, d - 1, c0:c1 + 1, :CB],
                            mb[:, :ncl, :CB],
                            taf[:, 7 * lo + d:7 * hi + d + 1:7, :CB], OP.add)

                    # own-anchor mining straight off the fp16 DP row
                    dtw16 = dpa[:, ND - 2, 8, :CB]
                    scr = mtmp.tile([128, CBMAX], fp32, tag="scr")
                    nc.vector.tensor_tensor(
                        scr[:, :CB], dtw16, mop[:, col0:col0 + CB], OP.add)
                    nc.vector.tensor_reduce(
                        apacc[:, n:n + 1], scr[:, :CB], axis=AX.X, op=OP.max)
                    nc.vector.tensor_tensor(
                        scr[:, :CB], dtw16, mon[:, col0:col0 + CB], OP.add)
                    nc.vector.tensor_reduce(
                        anacc[:, n:n + 1], scr[:, :CB], axis=AX.X, op=OP.min)

                emit_post(NB - 1)
                nc.vector.tensor_reduce(
                    pout[:, 0:1], anacc[:], axis=AX.X, op=OP.min)
                nc.vector.tensor_reduce(
                    pout[:, 1:2], apacc[:], axis=AX.X, op=OP.max)
                nc.sync.dma_start(out_t[:], pout[:])

    nc.compile()
    return nc


def _get_nc():
    if "nc" not in _CACHE:
        _CACHE["nc"] = _build_nc()
    return _CACHE["nc"]


def kernel(inputs, labels, _trace=False, _trace_cores=None):
    from concourse.bass_utils import run_bass_kernel_spmd

    x = np.ascontiguousarray(np.asarray(inputs, dtype=np.float32)).reshape(N * M, D)
    lab = np.asarray(labels)

    nc = _get_nc()
    in_maps = []
    for c in range(NCORES):
        blocks = [(c + d) % NCORES for d in range(NDIAG)]
        col_img = np.concatenate([np.arange(b * A, (b + 1) * A) for b in blocks])
        row_img = np.arange(c * A, (c + 1) * A)
        xr5 = np.ascontiguousarray(
            x.reshape(N, M, D)[col_img].reshape(NCOL * M, D))
        xa = np.ascontiguousarray(x[c * A * M:(c + 1) * A * M])
        eq_own = lab[row_img][:, None] == lab[col_img][None, :]
        m_own_pos = np.where(eq_own, np.float32(0.0), np.float32(-1e30))
        m_own_neg = np.where(eq_own, np.float32(1e30), np.float32(0.0))
        # transposed blocks: anchors = block (c+d)%8, cols = block c images
        mtp_l, mtn_l = [], []
        for d in range(1, NDIAG):
            arow = lab[np.arange(blocks[d] * A, (blocks[d] + 1) * A)]
            eq_t = arow[:, None] == lab[row_img][None, :]
            mtp_l.append(np.where(eq_t, np.float32(0.0), np.float32(-1e30)))
            mtn_l.append(np.where(eq_t, np.float32(1e30), np.float32(0.0)))
        in_maps.append({
            "xa": xa,
            "xr5": xr5,
            "m_own_pos": np.ascontiguousarray(m_own_pos.astype(np.float32)),
            "m_own_neg": np.ascontiguousarray(m_own_neg.astype(np.float32)),
            "m_t_pos": np.ascontiguousarray(
                np.concatenate(mtp_l, axis=1).astype(np.float32)),
            "m_t_neg": np.ascontiguousarray(
                np.concatenate(mtn_l, axis=1).astype(np.float32)),
        })
    res = run_bass_kernel_spmd(
        nc, in_maps, core_ids=list(range(NCORES)), trace=_trace,
        trace_cores=_trace_cores)
    if _trace:
        _CACHE["last_results"] = res

    # host glue: combine per-core min/max partials per anchor block, then
    # undo the affine t-fit (GAM; the offset cancels in ap-an)
    an_all = np.full((NCORES, A), np.inf, dtype=np.float32)
    ap_all = np.full((NCORES, A), -np.inf, dtype=np.float32)
    for c in range(NCORES):
        p = res.results[c]["partials"]  # [A, 10]
        for d in range(NDIAG):
            blk = (c + d) % NCORES
            an_all[blk] = np.minimum(an_all[blk], p[:, 2 * d])
            ap_all[blk] = np.maximum(ap_all[blk], p[:, 2 * d + 1])
    loss_vec = np.maximum(
        np.float32(GAM) * (ap_all.reshape(-1) - an_all.reshape(-1))
        + np.float32(MARGIN),
        np.float32(0.0))
    return np.asarray(loss_vec.mean(), dtype=np.float32)
